# revision 18
# baseline (speedup 1.0000x reference)
"""Multi-head attention kernel for Trainium2, sharded over 8 NeuronCores.

Problem: Q,K,V [4, 16, 2048, 128] fp32 -> softmax(Q K^T / sqrt(128)) V.

Sharding: the 4*16 = 64 (batch, head) pairs are split across 8 cores,
8 pairs per core (pure data parallelism, no collectives).

Per-core kernel (flash-attention style, S^T layout):
  - Q, K are DMA-cast fp32->fp16 and transposed on the PE into
    Qt/Kt [d=128, seq] layout; V is DMA-cast into V_aug [k, 129] tiles
    whose last column is 1.0 (so the PV matmul also produces row sums).
  - S^T[k, q] tiles = Kt_tile^T @ Qt_chunk accumulate in PSUM;
    ACT computes P^T = exp(S^T / sqrt(d)) (no max subtraction; scores
    are bounded ~|6| for N(0,1) inputs so fp32 exp is safe).
  - O_unnorm[q, 0:128] and row sums [q, 128] accumulate in PSUM over all
    k tiles via matmul(lhsT=P^T slice, rhs=V_aug).
  - Final normalize: O = O_unnorm * (1/sums) on the vector engine.
"""

import os
import sys

for _p in ("/opt/trn_rl_repo",):
    if _p not in sys.path and os.path.isdir(_p):
        sys.path.insert(0, _p)

import numpy as np

import concourse.bass as bass
import concourse.bacc as bacc
import concourse.tile as tile
from concourse import mybir
from concourse.bass_utils import run_bass_kernel_spmd
from concourse.masks import make_identity

F32 = mybir.dt.float32
F16 = mybir.dt.float16

B, H, S, D = 4, 16, 2048, 128
N_CORES = 8
PAIRS = (B * H) // N_CORES  # (b,h) pairs per core
P = 128  # partition dim / head dim / seq tile

_nc_cache = {}


def build_nc(pairs=PAIRS, seq=S):
    """Build the per-core Bass program (SPMD: same program on all cores)."""
    key = (pairs, seq)
    if key in _nc_cache:
        return _nc_cache[key]

    NT = seq // P          # seq tiles (16)
    W = 256                # q-chunk width (2 psum O accumulators)
    QC = seq // W          # q chunks (8)
    # k-tile groups per q chunk: exp ops as large as PSUM allows
    # (st slots are 2 banks = [128, 4*W] fp32, double-buffered)
    GK_MAX = 4
    groups = []
    k0 = 0
    while k0 < NT:
        g = min(GK_MAX, NT - k0)
        groups.append((k0, g))
        k0 += g
    DA = D + 1             # V augmented with a ones column
    SCALE = float(1.0 / np.sqrt(D))

    nc = bacc.Bacc("TRN2", target_bir_lowering=False, debug=False)
    Qd = nc.dram_tensor("Q", [pairs, seq, D], F32, kind="ExternalInput").ap()
    Kd = nc.dram_tensor("K", [pairs, seq, D], F32, kind="ExternalInput").ap()
    Vd = nc.dram_tensor("V", [pairs, seq, D], F32, kind="ExternalInput").ap()
    Od = nc.dram_tensor("O", [pairs, seq, D], F32, kind="ExternalOutput").ap()

    with tile.TileContext(nc) as tc:
        with (
            tc.tile_pool(name="consts", bufs=1) as consts,
            tc.tile_pool(name="ld32", bufs=2) as ld32_pool,
            tc.tile_pool(name="ld", bufs=2) as ld_pool,
            tc.tile_pool(name="tr", bufs=2) as tr_pool,
            tc.tile_pool(name="pt", bufs=3) as pt_pool,
            tc.tile_pool(name="ost", bufs=2) as ost_pool,
            tc.tile_pool(name="sm", bufs=4) as sm_pool,
            tc.tile_pool(name="st_ps", bufs=2, space="PSUM") as st_ps,
            tc.tile_pool(name="o_ps", bufs=1, space="PSUM") as o_ps,
            tc.tile_pool(name="tp_ps", bufs=2, space="PSUM") as tp_ps,
        ):
            ident = consts.tile([P, P], F16)
            make_identity(nc, ident)

            for i in range(pairs):
                # ---- load fp32 (two HWDGE rings), cast fp16, PE-transpose ----
                # chunked so the cast/transpose pipeline starts as soon as the
                # first 256KB lands instead of after the full 1MB
                NCH = 4
                CT = NT // NCH            # seq tiles per chunk
                CW = CT * P               # columns per chunk
                Qb32 = ld32_pool.tile([P, seq], F32, tag="Qb32", name=f"Qb32_{i}")
                Kb32 = ld32_pool.tile([P, seq], F32, tag="Kb32", name=f"Kb32_{i}")
                Vb32 = ld32_pool.tile([P, seq], F32, tag="Vb32", name=f"Vb32_{i}")
                Qb = ld_pool.tile([P, seq], F16, tag="Qb", name=f"Qb{i}")
                Kb = ld_pool.tile([P, seq], F16, tag="Kb", name=f"Kb{i}")
                Qt = tr_pool.tile([P, seq], F16, tag="Qt", name=f"Qt{i}")
                Kt = tr_pool.tile([P, seq], F16, tag="Kt", name=f"Kt{i}")
                Vaug = ld_pool.tile([P, NT * DA], F16, tag="Vaug", name=f"Vaug{i}")
                Vv = Vaug.rearrange("p (t e) -> p t e", e=DA)

                def load_chunk(dst32, src_dram, c, eng):
                    eng.dma_start(
                        out=dst32.rearrange("p (t d) -> p t d", d=P)[
                            :, c * CT : (c + 1) * CT
                        ],
                        in_=src_dram.rearrange("(t p) d -> p t d", p=P)[
                            :, c * CT : (c + 1) * CT
                        ],
                    )

                for c in range(NCH):
                    # K on the ACT ring, Q on the SP ring: parallel loads
                    load_chunk(Kb32, Kd[i], c, nc.scalar)
                    load_chunk(Qb32, Qd[i], c, nc.sync)
                    sl = slice(c * CW, (c + 1) * CW)
                    nc.vector.tensor_copy(out=Kb[:, sl], in_=Kb32[:, sl])
                    nc.vector.tensor_copy(out=Qb[:, sl], in_=Qb32[:, sl])
                    for src, dst in ((Kb, Kt), (Qb, Qt)):
                        for t in range(c * CT, (c + 1) * CT):
                            tp = tp_ps.tile(
                                [P, P], F16, tag="tp", name=f"tp{i}_{t}"
                            )
                            nc.tensor.transpose(
                                tp, src[:, t * P : (t + 1) * P], ident
                            )
                            nc.vector.tensor_copy(
                                out=dst[:, t * P : (t + 1) * P], in_=tp
                            )

                for c in range(NCH):
                    load_chunk(Vb32, Vd[i], c, nc.scalar)
                nc.gpsimd.tensor_copy(
                    out=Vv[:, :, 0:D],
                    in_=Vb32.rearrange("p (t d) -> p t d", d=P),
                )
                nc.vector.memset(Vv[:, :, D:DA], 1.0)

                Ost = ost_pool.tile([P, seq], F32, tag="Ost", name=f"Ost{i}")

                # ---- flash loop ----
                for qc in range(QC):
                    # one PSUM bank per O accumulator (own zero region)
                    o_acc = [
                        o_ps.tile([P, 512], F32, tag=f"o{qt}", name=f"o{i}_{qc}_{qt}")
                        for qt in range(W // P)
                    ]

                    def emit_pv(pt_tile, k0, gk):
                        for j in range(gk):
                            kt = k0 + j
                            for qt in range(W // P):
                                nc.tensor.matmul(
                                    o_acc[qt][:, 0:DA],
                                    lhsT=pt_tile[:, j * W + qt * P : j * W + (qt + 1) * P],
                                    rhs=Vaug[:, kt * DA : (kt + 1) * DA],
                                    start=(kt == 0),
                                    stop=(kt == NT - 1),
                                )

                    pending = None
                    for k0, gk in groups:
                        stp = st_ps.tile(
                            [P, GK_MAX * W], F32, tag="st", name=f"st{i}_{qc}_{k0}"
                        )
                        for j in range(gk):
                            kt = k0 + j
                            nc.tensor.matmul(
                                stp[:, j * W : (j + 1) * W],
                                lhsT=Kt[:, kt * P : (kt + 1) * P],
                                rhs=Qt[:, qc * W : (qc + 1) * W],
                                start=True,
                                stop=True,
                            )
                        pt = pt_pool.tile(
                            [P, GK_MAX * W], F16, tag="pt", name=f"pt{i}_{qc}_{k0}"
                        )
                        nc.scalar.activation(
                            out=pt[:, 0 : gk * W],
                            in_=stp[:, 0 : gk * W],
                            func=mybir.ActivationFunctionType.Exp,
                            bias=0.0,
                            scale=SCALE,
                        )
                        # software pipeline: PV of the previous group is
                        # emitted after this group's S^T matmuls so the PE
                        # never stalls waiting on ACT.
                        if pending is not None:
                            emit_pv(*pending)
                        pending = (pt, k0, gk)
                    emit_pv(*pending)

                    # ---- normalize: O = O_unnorm / row_sums ----
                    for qt in range(W // P):
                        t = qc * (W // P) + qt
                        rec = sm_pool.tile([P, 1], F32, tag="rec", name=f"rec{i}_{t}")
                        nc.vector.reciprocal(out=rec, in_=o_acc[qt][:, D : D + 1])
                        nc.vector.tensor_scalar_mul(
                            Ost[:, t * P : (t + 1) * P], o_acc[qt][:, 0:D], rec
                        )

                nc.sync.dma_start(
                    out=Od[i].rearrange("(t p) d -> p t d", p=P),
                    in_=Ost.rearrange("p (t d) -> p t d", d=P),
                )

    nc.compile()
    _nc_cache[key] = nc
    return nc


def run(Q, K, V, trace=False):
    """Run on 8 cores; Q/K/V are full [B,H,S,D] fp32 arrays.

    Returns (output [B,H,S,D] fp32, BassKernelResults)."""
    Qf = np.ascontiguousarray(np.asarray(Q, dtype=np.float32).reshape(B * H, S, D))
    Kf = np.ascontiguousarray(np.asarray(K, dtype=np.float32).reshape(B * H, S, D))
    Vf = np.ascontiguousarray(np.asarray(V, dtype=np.float32).reshape(B * H, S, D))

    nc = build_nc()
    in_maps = [
        {
            "Q": Qf[c * PAIRS : (c + 1) * PAIRS],
            "K": Kf[c * PAIRS : (c + 1) * PAIRS],
            "V": Vf[c * PAIRS : (c + 1) * PAIRS],
        }
        for c in range(N_CORES)
    ]
    res = run_bass_kernel_spmd(nc, in_maps, list(range(N_CORES)), trace=trace)
    out = np.concatenate([res.results[c]["O"] for c in range(N_CORES)], axis=0)
    return out.reshape(B, H, S, D), res


def kernel(Q, K, V):
    out, _ = run(Q, K, V, trace=False)
    return out
